# revision 30
# baseline (speedup 1.0000x reference)
"""Binarized MLP (784 -> 1024 -> 1024 -> 1024 -> 10) on 8 TRN2 NeuronCores.

Data-parallel over the batch (16384 rows -> 2048 per core), weights replicated.

Math notes:
  * Layers 1-2 outputs are only consumed through binarize(hardtanh(bn(h))).
    With g > 0, be == 0 that is exactly sign(h + (b - m)): one ScalarE Sign
    activation with a per-partition bias.
  * fc2/fc3 multiply two +-1 operands -> exact in fp8(e4m3) with fp32 PSUM
    accumulation; DoubleRow contracts 256 rows per 512-cycle pass.
  * fc1 keeps x near-full precision in 10 PE passes (vs 13 for exact hi/lo):
    7 f16 hi chunks (rows 768-783 ride chunk 6 as a packed hi/lo pair at
    p0-15/p16-31, zero elsewhere) plus an fp8 residual lo = x - f16(x)
    scaled by 2^12 contracted in 3 fp8-DR passes with weights sign*2^-6;
    the whole layer runs at a 2^6 global scale (hi weights sign*64, bias*64)
    so every fp8 value stays in the normal range. Sign() is scale-invariant.
    Host-simulated rel err: 9.4e-3 (22 act1 sign flips of 16.7M).
  * fc1 runs n-outer with PHASE-UNIFORM matmul modes: per 512-col block all
    56 f16 matmuls accumulate into 8 pinned PSUM banks, then all 24 fp8-DR
    matmuls close them. LDWEIGHTS only overlaps execution when the PE config
    (dtype/tile/perf-mode) matches, so interleaving modes costs ~260ns/tile.
  * ALL dram tensors are stored partition-major (host relayout) so every DMA
    is 128 descriptor rows of contiguous KBs: a `m k p c -> p m k c` gather
    costs ~2.4ns per 128-256B descriptor row on the issuing engine (a 1MB
    weight DMA = 7-9us of issue time, serializing its ring).
  * Batch columns are host-permuted (col c carries batch row 64*(c%32)+c//32)
    so after the DVE 32x32 output transpose each partition holds contiguous
    y rows: the output DMA is 32 x 640B runs instead of 2048 x 40B scatter.
  * fc4 + log_softmax: per n-block copy (DVE) / transpose / +b4 / Exp /
    reduce overlap later fc4 matmuls; the four Ln (a DIFFERENT activation
    table set than Exp!) run batched at the end so ScalarE swaps tables
    exactly twice in the tail instead of 8 times at 1.28us each.
"""

import os
import numpy as np

N_CORES = 8
B_FULL = 16384
BS = B_FULL // N_CORES  # 2048 rows per core
IN_F = 784
H = 1024
HC = 8                  # hidden chunks of 128
OUT_F = 10
NSPLIT = 4              # batch column blocks of 512
NB = BS // NSPLIT       # 512
KH = 7                  # f16 hi chunks (6 full + packed tail chunk)
KL = 6                  # fp8 lo chunks = 3 DoubleRow pairs (rows 0-767)
NJ = NB // 32           # 16 32-col blocks per n-block (output transpose)

LAST_RESULT = None      # BassKernelResults of the most recent run (for test.py)

_PLAN = {}


def _build_nc():
    import concourse.bass as bass
    import concourse.mybir as mybir
    import concourse.tile as tile
    from concourse.tile import add_dep_helper
    from concourse import bacc
    from concourse.bass import ts

    f32 = mybir.dt.float32
    f16 = mybir.dt.float16
    f8 = mybir.dt.float8e4
    AF = mybir.ActivationFunctionType
    ALU = mybir.AluOpType
    DR = mybir.MatmulPerfMode.DoubleRow

    nc = bacc.Bacc(None)

    # all layouts partition-major: leading dim 128 (or 32) is the SBUF
    # partition, everything after is contiguous per partition.
    xh_t = nc.dram_tensor("xh", [128, NSPLIT, KH, NB], f16, kind="ExternalInput")
    xl_t = nc.dram_tensor("xl", [128, NSPLIT, KL, NB], f8, kind="ExternalInput")
    # s1h values are sign*2^6 = +-64: exactly representable in e4m3, and a
    # mixed f16-moving x fp8-stationary matmul runs at full rate -> halves
    # the fc1 weight bytes on the DMA-bound ramp at zero accuracy cost.
    s1h_t = nc.dram_tensor("s1h", [128, HC, KH, 128], f8, kind="ExternalInput")
    s1l_t = nc.dram_tensor("s1l", [128, HC, KL, 128], f8, kind="ExternalInput")
    s2_t = nc.dram_tensor("s2t", [128, HC, HC, 128], f8, kind="ExternalInput")
    s3_t = nc.dram_tensor("s3t", [128, HC, HC, 128], f8, kind="ExternalInput")
    # w4 zero-padded to 32 output cols so fc4's PSUM is a full [32, NB]
    # tile the DVE 32x32 transpose can read directly (no copy, no uninit)
    w4_t = nc.dram_tensor("w4t", [128, HC, 32], f16, kind="ExternalInput")
    b1_t = nc.dram_tensor("bias1", [128, HC], f32, kind="ExternalInput")
    b2_t = nc.dram_tensor("bias2", [128, HC], f32, kind="ExternalInput")
    sc3_t = nc.dram_tensor("sc3", [128, HC], f32, kind="ExternalInput")
    sh3_t = nc.dram_tensor("sh3", [128, HC], f32, kind="ExternalInput")
    b4_t = nc.dram_tensor("b4", [OUT_F], f32, kind="ExternalInput")
    y_t = nc.dram_tensor("y", [BS, OUT_F], f32, kind="ExternalOutput")

    with tile.TileContext(nc) as tc:
        with (
            tc.tile_pool(name="consts", bufs=1) as consts,
            tc.tile_pool(name="tmp", bufs=4) as tmp,
            tc.tile_pool(name="psum", bufs=8, space="PSUM") as psum,
        ):
            xh_sb = consts.tile([128, NSPLIT, KH, NB], f16, tag="xh")
            xl_sb = consts.tile([128, NSPLIT, KL, NB], f8, tag="xl")
            s1h_sb = consts.tile([128, HC, KH, 128], f8, tag="s1h")
            s1l_sb = consts.tile([128, HC, KL, 128], f8, tag="s1l")
            s2_sb = consts.tile([128, HC, HC, 128], f8, tag="s2")
            s3_sb = consts.tile([128, HC, HC, 128], f8, tag="s3")
            w4_sb = consts.tile([128, HC, 32], f16, tag="w4")
            b1v = consts.tile([128, HC], f32, tag="b1v")
            b2v = consts.tile([128, HC], f32, tag="b2v")
            sc3v = consts.tile([128, HC], f32, tag="sc3v")
            sh3v = consts.tile([128, HC], f32, tag="sh3v")
            b4bc = consts.tile([32, OUT_F], f32, tag="b4bc")
            act1 = consts.tile([128, HC, BS], f8, tag="act1")
            act2 = consts.tile([128, HC, BS], f8, tag="act2")
            act3 = consts.tile([128, HC, BS], f16, tag="act3")
            ltr = consts.tile([32, BS], f32, tag="ltr")
            es = consts.tile([32, NSPLIT * NJ, OUT_F], f32, tag="es")
            lse = consts.tile([32, NSPLIT * NJ], f32, tag="lse")
            outf = consts.tile([32, NSPLIT * NJ, OUT_F], f32, tag="outf")
            warm = consts.tile([128, NB], f16, tag="warm")
            warm8 = consts.tile([128, 128], f8, tag="warm8")

            # ---- ramp DMAs, priority order. Three rings (sync/scalar HWDGE
            # + vector) split the fc1-n0 critical path: per-ring descriptor
            # feed caps ~150GB/s, so parallel rings = real bandwidth. The
            # gpsimd SWDGE ring (starts ~4us late) carries late-needed data.
            # warm-up tiles first so PE clock ramping can start ASAP
            nc.vector.memset(warm, 0.0)
            nc.vector.memset(warm8, 0.0)
            # tail zeros for n0 (partitions 32-127 of hi chunk 6) come from a
            # DVE memset so only partitions 0-31 of n0's chunk 6 transfer;
            # n1-3 take the zeros from DRAM (bandwidth is free by then).
            for p0 in (32, 64, 96):
                nc.vector.memset(xh_sb[p0:p0 + 32, 0:1, 6, :], 0.0)
            # need-order for n0's k-outer hi phase: m0 weights + chunk 0
            # first, then remaining weights, then chunks in k order.
            nc.sync.dma_start(out=s1h_sb[:, 0:1], in_=s1h_t[:, 0:1])
            nc.scalar.dma_start(out=s1h_sb[:, 1:2], in_=s1h_t[:, 1:2])
            nc.sync.dma_start(out=xh_sb[:, 0, 0:1], in_=xh_t[:, 0, 0:1])
            nc.scalar.dma_start(out=s1h_sb[:, 4:HC], in_=s1h_t[:, 4:HC])
            nc.sync.dma_start(out=s1h_sb[:, 2:4], in_=s1h_t[:, 2:4])
            nc.scalar.dma_start(out=xh_sb[:, 0, 1:2], in_=xh_t[:, 0, 1:2])
            nc.sync.dma_start(out=xh_sb[:, 0, 2:3], in_=xh_t[:, 0, 2:3])
            nc.scalar.dma_start(out=xh_sb[:, 0, 3:4], in_=xh_t[:, 0, 3:4])
            nc.sync.dma_start(out=xh_sb[:, 0, 4:5], in_=xh_t[:, 0, 4:5])
            nc.scalar.dma_start(out=xh_sb[:, 0, 5:6], in_=xh_t[:, 0, 5:6])
            nc.sync.dma_start(out=xh_sb[0:32, 0, 6:KH], in_=xh_t[0:32, 0, 6:KH])
            nc.scalar.dma_start(out=s1l_sb[:, 0:4], in_=s1l_t[:, 0:4])
            nc.gpsimd.dma_start(out=b1v, in_=b1_t[:])
            # bulk needed only mid-fc1 or later: gated behind fc1 progress
            # markers so the shared DMA-engine pool (per-packet round-robin,
            # no priority) is never stolen from the next-needed transfers.
            d_xl0 = nc.gpsimd.dma_start(out=xl_sb[:, 0], in_=xl_t[:, 0])
            d_s1l = nc.gpsimd.dma_start(out=s1l_sb[:, 4:HC], in_=s1l_t[:, 4:HC])
            d_xl1 = nc.gpsimd.dma_start(out=xl_sb[:, 1], in_=xl_t[:, 1])
            nc.sync.dma_start(out=xh_sb[:, 1], in_=xh_t[:, 1])
            d_xl2 = nc.sync.dma_start(out=xl_sb[:, 2], in_=xl_t[:, 2])
            d_xh3 = nc.sync.dma_start(out=xh_sb[:, 3], in_=xh_t[:, 3])
            d_xh2 = nc.scalar.dma_start(out=xh_sb[:, 2], in_=xh_t[:, 2])
            d_xl3 = nc.scalar.dma_start(out=xl_sb[:, 3], in_=xl_t[:, 3])
            # small later-layer params (trivial bandwidth)
            nc.gpsimd.dma_start(out=w4_sb, in_=w4_t[:])
            nc.gpsimd.dma_start(out=b2v, in_=b2_t[:])
            nc.gpsimd.dma_start(out=sc3v, in_=sc3_t[:])
            nc.gpsimd.dma_start(out=sh3v, in_=sh3_t[:])
            b4_ap = b4_t[:]
            nc.gpsimd.dma_start(
                out=b4bc,
                in_=bass.AP(tensor=b4_ap.tensor, offset=b4_ap.offset,
                            ap=[[0, 32]] + list(b4_ap.ap)),
            )

            # PE clock warm-up: the tensor engine runs at ~1.2GHz until ~3us
            # of continuous execution. Dummy matmuls on zeroed tiles (same
            # fp8-stationary/f16-moving config as fc1's hi phase) spin the
            # clock up during the DMA ramp when the PE would idle anyway.
            pw = psum.tile([128, NB], f32, tag="mm", name="warm")
            for i in range(10):
                nc.tensor.matmul(pw, warm8, warm, start=(i == 0), stop=(i == 9))

            # ---- fc1: phase-uniform per n-block; 8 PSUM banks pinned.
            # h1*2^6 = hi-chunk contractions (f16) then lo residual (fp8 DR),
            # Sign -> act1.
            x_gate = None
            for n in range(NSPLIT):
                pss = [psum.tile([128, NB], f32, tag="mm", name="ps") for _ in range(HC)]
                if n == 0:
                    # k-outer: consumes one x chunk per 8 matmuls (1.7us),
                    # matching the throttled early DMA arrival rate instead
                    # of demanding all 7 chunks in the first 1.5us.
                    for k in range(KH):
                        for m in range(HC):
                            mmv = nc.tensor.matmul(
                                pss[m], s1h_sb[:, m, k], xh_sb[:, n, k],
                                start=(k == 0), stop=False,
                            )
                            if m == 0:
                                g = {2: d_xl0, 3: d_s1l, 5: d_xl1}.get(k)
                                if g is not None:
                                    add_dep_helper(g.ins, mmv.ins,
                                                   reason="defer bulk dma")
                else:
                    for m in range(HC):
                        for k in range(KH):
                            mmv = nc.tensor.matmul(
                                pss[m], s1h_sb[:, m, k], xh_sb[:, n, k],
                                start=(k == 0), stop=False,
                            )
                            if k == 0:
                                g = {(1, 0): d_xh2, (1, 2): d_xh3,
                                     (1, 4): d_xl2, (2, 0): d_xl3}.get((n, m))
                                if g is not None:
                                    add_dep_helper(g.ins, mmv.ins,
                                                   reason="defer bulk dma")
                for m in range(HC):
                    for j in range(KL // 2):
                        jsl = slice(2 * j, 2 * j + 2)
                        nc.tensor.matmul(
                            pss[m], s1l_sb[:, m, jsl], xl_sb[:, n, jsl],
                            start=False, stop=(j == KL // 2 - 1),
                            perf_mode=DR,
                        )
                    a = nc.scalar.activation(
                        act1[:, m, ts(n, NB)], pss[m], AF.Sign, bias=b1v[:, m:m + 1]
                    )
                    if n == 1 and m == 0:
                        x_gate = a

            # later-layer weights deferred until fc1 is past the DMA crunch
            for a0 in (0, 4):
                d = nc.gpsimd.dma_start(out=s2_sb[:, a0:a0 + 4], in_=s2_t[:, a0:a0 + 4])
                add_dep_helper(d.ins, x_gate.ins, reason="defer s2 after x load")
            for a0 in (0, 4):
                d = nc.gpsimd.dma_start(out=s3_sb[:, a0:a0 + 4], in_=s3_t[:, a0:a0 + 4])
                add_dep_helper(d.ins, x_gate.ins, reason="defer s3 after x load")

            # ---- fc2: binary x binary, fp8 DoubleRow, sign -> act2 ----
            for m in range(HC):
                pss = [psum.tile([128, NB], f32, tag="mm", name="ps") for _ in range(NSPLIT)]
                for kk in range(HC // 2):
                    ksl = slice(2 * kk, 2 * kk + 2)
                    for n in range(NSPLIT):
                        nc.tensor.matmul(
                            pss[n], s2_sb[:, m, ksl], act1[:, ksl, ts(n, NB)],
                            start=(kk == 0), stop=(kk == HC // 2 - 1),
                            perf_mode=DR,
                        )
                for n in range(NSPLIT):
                    nc.scalar.activation(
                        act2[:, m, ts(n, NB)], pss[n], AF.Sign, bias=b2v[:, m:m + 1]
                    )

            # ---- fc3: fp8 DoubleRow, bn affine + hardtanh -> act3 (DVE) ----
            for m in range(HC):
                pss = [psum.tile([128, NB], f32, tag="mm", name="ps") for _ in range(NSPLIT)]
                for kk in range(HC // 2):
                    ksl = slice(2 * kk, 2 * kk + 2)
                    for n in range(NSPLIT):
                        nc.tensor.matmul(
                            pss[n], s3_sb[:, m, ksl], act2[:, ksl, ts(n, NB)],
                            start=(kk == 0), stop=(kk == HC // 2 - 1),
                            perf_mode=DR,
                        )
                for n in range(NSPLIT):
                    t = tmp.tile([128, NB], f32, tag="t3")
                    nc.scalar.activation(
                        t, pss[n], AF.Identity,
                        bias=sh3v[:, m:m + 1], scale=sc3v[:, m:m + 1],
                    )
                    nc.vector.tensor_scalar(
                        out=act3[:, m, ts(n, NB)], in0=t,
                        scalar1=-1.0, scalar2=1.0,
                        op0=ALU.max, op1=ALU.min,
                    )

            # ---- fc4 + log_softmax ----
            # ltr[p, 32u+o] = logit class o of column 32u+p; column 32u+p
            # carries batch row 64p+u (host permutation), so partition p of
            # outf holds y rows 64p..64p+63 contiguously. Phase 1 per n
            # (through Exp/reduce) overlaps later fc4 matmuls; the Ln's
            # (different act table than Exp) run batched at the end.
            yb = y_t[:]
            b4r = b4bc[:]
            NB4 = 256                     # fc4 column block (shorter tail)
            NJ4 = NB4 // 32
            for nn in range(BS // NB4):
                ps4 = psum.tile([32, NB4], f32, tag="mm", name="ps4")
                for k in range(HC):
                    nc.tensor.matmul(
                        ps4, w4_sb[:, k], act3[:, k, ts(nn, NB4)],
                        start=(k == 0), stop=(k == HC - 1),
                    )
                nc.vector.transpose(ltr[:, ts(nn, NB4)], ps4)
                base = ltr[:, ts(nn, NB4)]
                ltv = bass.AP(tensor=base.tensor, offset=base.offset,
                              ap=[base.ap[0], [32, NJ4], [1, OUT_F]])
                nc.vector.tensor_tensor(
                    out=ltv, in0=ltv,
                    in1=bass.AP(tensor=b4r.tensor, offset=b4r.offset,
                                ap=[[b4r.ap[0][0], 32], [0, NJ4], b4r.ap[1]]),
                    op=ALU.add,
                )
                nc.scalar.activation(es[:, ts(nn, NJ4), :], ltv, AF.Exp)
                nc.vector.tensor_reduce(
                    out=lse[:, ts(nn, NJ4)], in_=es[:, ts(nn, NJ4), :],
                    axis=mybir.AxisListType.X, op=ALU.add,
                )
            nc.scalar.activation(lse, lse, AF.Ln)
            # one full-width subtract + one contiguous y DMA (32 x 2560B)
            lall = ltr[:]
            ltv_all = bass.AP(tensor=lall.tensor, offset=lall.offset,
                              ap=[lall.ap[0], [32, NSPLIT * NJ], [1, OUT_F]])
            lser = lse[:]
            nc.vector.tensor_tensor(
                out=outf, in0=ltv_all,
                in1=bass.AP(tensor=lser.tensor, offset=lser.offset,
                            ap=[lser.ap[0], lser.ap[1], [0, OUT_F]]),
                op=ALU.subtract,
            )
            nc.sync.dma_start(
                out=bass.AP(tensor=yb.tensor, offset=yb.offset,
                            ap=[[64 * OUT_F, 32], [OUT_F, NSPLIT * NJ],
                                [1, OUT_F]]),
                in_=outf,
            )

    nc.finalize()
    return nc


def _host_prep(inputs):
    """Shard x, binarize/lay out weights (partition-major), fold bn biases."""
    import ml_dtypes

    f16 = np.float16
    f8 = ml_dtypes.float8_e4m3

    x = np.asarray(inputs["x"], np.float32)
    w1 = np.asarray(inputs["w1"], np.float32)
    w2 = np.asarray(inputs["w2"], np.float32)
    w3 = np.asarray(inputs["w3"], np.float32)
    w4 = np.asarray(inputs["w4"], np.float32)
    b1 = np.asarray(inputs["b1"], np.float32)
    b2 = np.asarray(inputs["b2"], np.float32)
    b3 = np.asarray(inputs["b3"], np.float32)
    b4 = np.asarray(inputs["b4"], np.float32)

    EPS = np.float64(1e-5)

    def gv(i):
        return (np.asarray(inputs[f"g{i}"], np.float32),
                np.asarray(inputs[f"be{i}"], np.float32),
                np.asarray(inputs[f"m{i}"], np.float32),
                np.asarray(inputs[f"v{i}"], np.float32))

    g1, be1, m1, v1 = gv(1)
    g2, be2, m2, v2 = gv(2)
    g3, be3, m3, v3 = gv(3)
    # sign(bn(h)) == sign(h + (b - m)) requires gamma > 0 and beta == 0
    assert np.all(g1 > 0) and np.all(be1 == 0), "unsupported bn1 params"
    assert np.all(g2 > 0) and np.all(be2 == 0), "unsupported bn2 params"

    def pmaj(v):  # [1024] -> [128, 8] partition-major
        return np.ascontiguousarray(v.reshape(HC, 128).T)

    bias1 = pmaj(((b1 - m1) * 64.0).astype(np.float32))  # fc1 runs at 2^6
    bias2 = pmaj((b2 - m2).astype(np.float32))
    r3 = 1.0 / np.sqrt(v3.astype(np.float64) + EPS)
    sc3 = pmaj((r3 * g3).astype(np.float32))
    sh3 = pmaj(((b3 - m3).astype(np.float64) * r3 * g3 + be3).astype(np.float32))

    # fc1 weights: rows 0-767 split [p][m][k][c]; hi at sign*2^6 (f16),
    # lo at sign*2^-6 (fp8, min normal). Hi chunk 6 is the packed tail:
    # rows 768-783 replicated at p0-15 and p16-31 (hi/lo), zero elsewhere.
    s1f = np.sign(w1).T.astype(np.float32)              # [784, 1024]
    body = s1f[:768].reshape(6, 128, HC, 128).transpose(2, 0, 1, 3)  # [m,k,p,c]
    s1h = np.zeros((HC, KH, 128, 128), np.float32)
    s1h[:, :6] = body * 64.0
    tail = s1f[768:IN_F].reshape(16, HC, 128) * 64.0    # [16, 8, 128]
    for mm in range(HC):
        s1h[mm, 6, 0:16] = tail[:, mm]
        s1h[mm, 6, 16:32] = tail[:, mm]
    s1h = np.ascontiguousarray(s1h.transpose(2, 0, 1, 3)).astype(f8)  # [p,m,k,c]
    s1l = np.ascontiguousarray(
        (body * (2.0 ** -6)).transpose(2, 0, 1, 3)).astype(f8)         # [p,m,k,c]

    def wlay(w, dt):  # [out, in] -> [p, m, k, c] partition-major
        st = np.sign(w).T.astype(np.float32)            # [in, out]
        a = st.reshape(HC, 128, HC, 128).transpose(2, 0, 1, 3)  # [m,k,p,c]
        return np.ascontiguousarray(a.transpose(2, 0, 1, 3)).astype(dt)

    s2t = wlay(w2, f8)
    s3t = wlay(w3, f8)
    w4p = np.zeros((H, 32), np.float32)                 # zero-pad 10 -> 32
    w4p[:, :OUT_F] = w4.T
    w4t = np.ascontiguousarray(
        w4p.astype(f16).reshape(HC, 128, 32).transpose(1, 0, 2))  # [p,m,o]

    shared = dict(s1h=s1h, s1l=s1l, s2t=s2t, s3t=s3t, w4t=w4t,
                  bias1=bias1, bias2=bias2, sc3=sc3, sh3=sh3, b4=b4)

    # column permutation: kernel column c carries batch row 64*(c%32)+c//32,
    # so the transposed output lands contiguously per partition.
    cc = np.arange(BS)
    perm = 64 * (cc % 32) + cc // 32

    in_maps = []
    for c in range(N_CORES):
        xs = x[c * BS:(c + 1) * BS][perm]               # [2048, 784] permuted
        xcols = np.ascontiguousarray(xs.T)              # [784, 2048] fp32
        xhi = xcols.astype(f16)
        lo32 = xcols - xhi.astype(np.float32)           # exact residual
        xh = np.zeros((KH, 128, BS), f16)
        xh[:6] = xhi[:768].reshape(6, 128, BS)
        xh[6, 0:16] = xhi[768:IN_F]
        xh[6, 16:32] = lo32[768:IN_F].astype(f16)
        # [k, p, (n nb)] -> [p, n, k, nb]
        xh = np.ascontiguousarray(
            xh.reshape(KH, 128, NSPLIT, NB).transpose(1, 2, 0, 3))
        xl = np.ascontiguousarray(
            (lo32[:768] * 4096.0).reshape(KL, 128, NSPLIT, NB)
            .transpose(1, 2, 0, 3)).astype(f8)
        m = dict(shared)
        m["xh"] = xh
        m["xl"] = xl
        in_maps.append(m)
    return in_maps


def kernel(**inputs):
    global LAST_RESULT
    from concourse.bass_utils import run_bass_kernel_spmd

    if "nc" not in _PLAN:
        _PLAN["nc"] = _build_nc()
    nc = _PLAN["nc"]

    in_maps = _host_prep(inputs)
    br = run_bass_kernel_spmd(
        nc, in_maps, list(range(N_CORES)),
        tmpdir=os.environ.get("KERNEL_TMPDIR") or None,
    )
    LAST_RESULT = br
    out = np.concatenate([br.results[c]["y"] for c in range(N_CORES)], axis=0)
    return out.astype(np.float32)


# revision 31
# speedup vs baseline: 1.0051x; 1.0051x over previous
"""Binarized MLP (784 -> 1024 -> 1024 -> 1024 -> 10) on 8 TRN2 NeuronCores.

Data-parallel over the batch (16384 rows -> 2048 per core), weights replicated.

Math notes:
  * Layers 1-2 outputs are only consumed through binarize(hardtanh(bn(h))).
    With g > 0, be == 0 that is exactly sign(h + (b - m)): one ScalarE Sign
    activation with a per-partition bias.
  * fc2/fc3 multiply two +-1 operands -> exact in fp8(e4m3) with fp32 PSUM
    accumulation; DoubleRow contracts 256 rows per 512-cycle pass.
  * fc1 keeps x near-full precision in 10 PE passes (vs 13 for exact hi/lo):
    7 f16 hi chunks (rows 768-783 ride chunk 6 as a packed hi/lo pair at
    p0-15/p16-31, zero elsewhere) plus an fp8 residual lo = x - f16(x)
    scaled by 2^12 contracted in 3 fp8-DR passes with weights sign*2^-6;
    the whole layer runs at a 2^6 global scale (hi weights sign*64, bias*64)
    so every fp8 value stays in the normal range. Sign() is scale-invariant.
    Host-simulated rel err: 9.4e-3 (22 act1 sign flips of 16.7M).
  * fc1 runs n-outer with PHASE-UNIFORM matmul modes: per 512-col block all
    56 f16 matmuls accumulate into 8 pinned PSUM banks, then all 24 fp8-DR
    matmuls close them. LDWEIGHTS only overlaps execution when the PE config
    (dtype/tile/perf-mode) matches, so interleaving modes costs ~260ns/tile.
  * ALL dram tensors are stored partition-major (host relayout) so every DMA
    is 128 descriptor rows of contiguous KBs: a `m k p c -> p m k c` gather
    costs ~2.4ns per 128-256B descriptor row on the issuing engine (a 1MB
    weight DMA = 7-9us of issue time, serializing its ring).
  * Batch columns are host-permuted (col c carries batch row 64*(c%32)+c//32)
    so after the DVE 32x32 output transpose each partition holds contiguous
    y rows: the output DMA is 32 x 640B runs instead of 2048 x 40B scatter.
  * fc4 + log_softmax: per n-block copy (DVE) / transpose / +b4 / Exp /
    reduce overlap later fc4 matmuls; the four Ln (a DIFFERENT activation
    table set than Exp!) run batched at the end so ScalarE swaps tables
    exactly twice in the tail instead of 8 times at 1.28us each.
"""

import os
import numpy as np

N_CORES = 8
B_FULL = 16384
BS = B_FULL // N_CORES  # 2048 rows per core
IN_F = 784
H = 1024
HC = 8                  # hidden chunks of 128
OUT_F = 10
NSPLIT = 4              # batch column blocks of 512
NB = BS // NSPLIT       # 512
KH = 7                  # f16 hi chunks (6 full + packed tail chunk)
KL = 6                  # fp8 lo chunks = 3 DoubleRow pairs (rows 0-767)
NJ = NB // 32           # 16 32-col blocks per n-block (output transpose)

LAST_RESULT = None      # BassKernelResults of the most recent run (for test.py)

_PLAN = {}


def _build_nc():
    import concourse.bass as bass
    import concourse.mybir as mybir
    import concourse.tile as tile
    from concourse.tile import add_dep_helper
    from concourse import bacc
    from concourse.bass import ts

    f32 = mybir.dt.float32
    f16 = mybir.dt.float16
    f8 = mybir.dt.float8e4
    AF = mybir.ActivationFunctionType
    ALU = mybir.AluOpType
    DR = mybir.MatmulPerfMode.DoubleRow

    nc = bacc.Bacc(None)

    # all layouts partition-major: leading dim 128 (or 32) is the SBUF
    # partition, everything after is contiguous per partition.
    xh_t = nc.dram_tensor("xh", [128, NSPLIT, KH, NB], f16, kind="ExternalInput")
    xl_t = nc.dram_tensor("xl", [128, NSPLIT, KL, NB], f8, kind="ExternalInput")
    # s1h values are sign*2^6 = +-64: exactly representable in e4m3, and a
    # mixed f16-moving x fp8-stationary matmul runs at full rate -> halves
    # the fc1 weight bytes on the DMA-bound ramp at zero accuracy cost.
    s1h_t = nc.dram_tensor("s1h", [128, HC, KH, 128], f8, kind="ExternalInput")
    s1l_t = nc.dram_tensor("s1l", [128, HC, KL, 128], f8, kind="ExternalInput")
    s2_t = nc.dram_tensor("s2t", [128, HC, HC, 128], f8, kind="ExternalInput")
    s3_t = nc.dram_tensor("s3t", [128, HC, HC, 128], f8, kind="ExternalInput")
    # w4 zero-padded to 32 output cols so fc4's PSUM is a full [32, NB]
    # tile the DVE 32x32 transpose can read directly (no copy, no uninit)
    w4_t = nc.dram_tensor("w4t", [128, HC, 32], f16, kind="ExternalInput")
    b1_t = nc.dram_tensor("bias1", [128, HC], f32, kind="ExternalInput")
    b2_t = nc.dram_tensor("bias2", [128, HC], f32, kind="ExternalInput")
    sc3_t = nc.dram_tensor("sc3", [128, HC], f32, kind="ExternalInput")
    sh3_t = nc.dram_tensor("sh3", [128, HC], f32, kind="ExternalInput")
    b4_t = nc.dram_tensor("b4", [OUT_F], f32, kind="ExternalInput")
    y_t = nc.dram_tensor("y", [BS, OUT_F], f32, kind="ExternalOutput")

    with tile.TileContext(nc) as tc:
        with (
            tc.tile_pool(name="consts", bufs=1) as consts,
            tc.tile_pool(name="tmp", bufs=4) as tmp,
            tc.tile_pool(name="psum", bufs=8, space="PSUM") as psum,
        ):
            xh_sb = consts.tile([128, NSPLIT, KH, NB], f16, tag="xh")
            xl_sb = consts.tile([128, NSPLIT, KL, NB], f8, tag="xl")
            s1h_sb = consts.tile([128, HC, KH, 128], f8, tag="s1h")
            s1l_sb = consts.tile([128, HC, KL, 128], f8, tag="s1l")
            s2_sb = consts.tile([128, HC, HC, 128], f8, tag="s2")
            s3_sb = consts.tile([128, HC, HC, 128], f8, tag="s3")
            w4_sb = consts.tile([128, HC, 32], f16, tag="w4")
            b1v = consts.tile([128, HC], f32, tag="b1v")
            b2v = consts.tile([128, HC], f32, tag="b2v")
            sc3v = consts.tile([128, HC], f32, tag="sc3v")
            sh3v = consts.tile([128, HC], f32, tag="sh3v")
            b4bc = consts.tile([32, OUT_F], f32, tag="b4bc")
            act1 = consts.tile([128, HC, BS], f8, tag="act1")
            act2 = consts.tile([128, HC, BS], f8, tag="act2")
            act3 = consts.tile([128, HC, BS], f16, tag="act3")
            ltr = consts.tile([32, BS], f32, tag="ltr")
            es = consts.tile([32, NSPLIT * NJ, OUT_F], f32, tag="es")
            lse = consts.tile([32, NSPLIT * NJ], f32, tag="lse")
            outf = consts.tile([32, NSPLIT * NJ, OUT_F], f32, tag="outf")
            warm = consts.tile([128, NB], f16, tag="warm")
            warm8 = consts.tile([128, 128], f8, tag="warm8")

            # ---- ramp DMAs, priority order. Three rings (sync/scalar HWDGE
            # + vector) split the fc1-n0 critical path: per-ring descriptor
            # feed caps ~150GB/s, so parallel rings = real bandwidth. The
            # gpsimd SWDGE ring (starts ~4us late) carries late-needed data.
            # warm-up tiles first so PE clock ramping can start ASAP
            nc.vector.memset(warm, 0.0)
            nc.vector.memset(warm8, 0.0)
            # tail zeros for n0 (partitions 32-127 of hi chunk 6) come from a
            # DVE memset so only partitions 0-31 of n0's chunk 6 transfer;
            # n1-3 take the zeros from DRAM (bandwidth is free by then).
            for p0 in (32, 64, 96):
                nc.vector.memset(xh_sb[p0:p0 + 32, 0:1, 6, :], 0.0)
            nc.sync.dma_start(out=s1h_sb[:, 0:1], in_=s1h_t[:, 0:1])
            nc.scalar.dma_start(out=xh_sb[:, 0, 1:2], in_=xh_t[:, 0, 1:2])
            nc.sync.dma_start(out=xh_sb[:, 0, 0:1], in_=xh_t[:, 0, 0:1])
            nc.scalar.dma_start(out=xh_sb[:, 0, 3:4], in_=xh_t[:, 0, 3:4])
            nc.sync.dma_start(out=xh_sb[:, 0, 2:3], in_=xh_t[:, 0, 2:3])
            nc.scalar.dma_start(out=s1h_sb[:, 1:2], in_=s1h_t[:, 1:2])
            nc.sync.dma_start(out=xh_sb[:, 0, 4:5], in_=xh_t[:, 0, 4:5])
            nc.scalar.dma_start(out=xh_sb[0:32, 0, 6:KH], in_=xh_t[0:32, 0, 6:KH])
            nc.sync.dma_start(out=xh_sb[:, 0, 5:6], in_=xh_t[:, 0, 5:6])
            nc.scalar.dma_start(out=s1h_sb[:, 2:4], in_=s1h_t[:, 2:4])
            nc.sync.dma_start(out=s1h_sb[:, 4:6], in_=s1h_t[:, 4:6])
            nc.scalar.dma_start(out=s1h_sb[:, 6:HC], in_=s1h_t[:, 6:HC])
            nc.scalar.dma_start(out=s1l_sb[:, 0:4], in_=s1l_t[:, 0:4])
            nc.gpsimd.dma_start(out=b1v, in_=b1_t[:])
            # bulk needed only mid-fc1 or later: gated behind fc1 progress
            # markers so the shared DMA-engine pool (per-packet round-robin,
            # no priority) is never stolen from the next-needed transfers.
            d_xl0 = nc.gpsimd.dma_start(out=xl_sb[:, 0], in_=xl_t[:, 0])
            d_s1l = nc.gpsimd.dma_start(out=s1l_sb[:, 4:HC], in_=s1l_t[:, 4:HC])
            d_xl1 = nc.gpsimd.dma_start(out=xl_sb[:, 1], in_=xl_t[:, 1])
            nc.sync.dma_start(out=xh_sb[:, 1], in_=xh_t[:, 1])
            d_xl2 = nc.sync.dma_start(out=xl_sb[:, 2], in_=xl_t[:, 2])
            d_xh3 = nc.sync.dma_start(out=xh_sb[:, 3], in_=xh_t[:, 3])
            d_xh2 = nc.scalar.dma_start(out=xh_sb[:, 2], in_=xh_t[:, 2])
            d_xl3 = nc.scalar.dma_start(out=xl_sb[:, 3], in_=xl_t[:, 3])
            # small later-layer params (trivial bandwidth)
            nc.gpsimd.dma_start(out=w4_sb, in_=w4_t[:])
            nc.gpsimd.dma_start(out=b2v, in_=b2_t[:])
            nc.gpsimd.dma_start(out=sc3v, in_=sc3_t[:])
            nc.gpsimd.dma_start(out=sh3v, in_=sh3_t[:])
            b4_ap = b4_t[:]
            nc.gpsimd.dma_start(
                out=b4bc,
                in_=bass.AP(tensor=b4_ap.tensor, offset=b4_ap.offset,
                            ap=[[0, 32]] + list(b4_ap.ap)),
            )

            # PE clock warm-up: the tensor engine runs at ~1.2GHz until ~3us
            # of continuous execution. Dummy matmuls on zeroed tiles (same
            # fp8-stationary/f16-moving config as fc1's hi phase) spin the
            # clock up during the DMA ramp when the PE would idle anyway.
            pw = psum.tile([128, NB], f32, tag="mm", name="warm")
            for i in range(10):
                nc.tensor.matmul(pw, warm8, warm, start=(i == 0), stop=(i == 9))

            # ---- fc1: phase-uniform per n-block; 8 PSUM banks pinned.
            # h1*2^6 = hi-chunk contractions (f16) then lo residual (fp8 DR),
            # Sign -> act1.
            x_gate = None
            for n in range(NSPLIT):
                pss = [psum.tile([128, NB], f32, tag="mm", name="ps") for _ in range(HC)]
                for m in range(HC):
                    for k in range(KH):
                        mmv = nc.tensor.matmul(
                            pss[m], s1h_sb[:, m, k], xh_sb[:, n, k],
                            start=(k == 0), stop=False,
                        )
                        if k == 0:
                            g = {(0, 1): d_xl0, (0, 2): d_s1l, (0, 6): d_xl1,
                                 (1, 0): d_xh2, (1, 2): d_xh3, (1, 4): d_xl2,
                                 (2, 0): d_xl3}.get((n, m))
                            if g is not None:
                                add_dep_helper(g.ins, mmv.ins,
                                               reason="defer bulk dma")
                for m in range(HC):
                    for j in range(KL // 2):
                        jsl = slice(2 * j, 2 * j + 2)
                        nc.tensor.matmul(
                            pss[m], s1l_sb[:, m, jsl], xl_sb[:, n, jsl],
                            start=False, stop=(j == KL // 2 - 1),
                            perf_mode=DR,
                        )
                    a = nc.scalar.activation(
                        act1[:, m, ts(n, NB)], pss[m], AF.Sign, bias=b1v[:, m:m + 1]
                    )
                    if n == 1 and m == 0:
                        x_gate = a

            # later-layer weights deferred until fc1 is past the DMA crunch
            for a0 in (0, 4):
                d = nc.gpsimd.dma_start(out=s2_sb[:, a0:a0 + 4], in_=s2_t[:, a0:a0 + 4])
                add_dep_helper(d.ins, x_gate.ins, reason="defer s2 after x load")
            for a0 in (0, 4):
                d = nc.gpsimd.dma_start(out=s3_sb[:, a0:a0 + 4], in_=s3_t[:, a0:a0 + 4])
                add_dep_helper(d.ins, x_gate.ins, reason="defer s3 after x load")

            # ---- fc2: binary x binary, fp8 DoubleRow, sign -> act2 ----
            for m in range(HC):
                pss = [psum.tile([128, NB], f32, tag="mm", name="ps") for _ in range(NSPLIT)]
                for kk in range(HC // 2):
                    ksl = slice(2 * kk, 2 * kk + 2)
                    for n in range(NSPLIT):
                        nc.tensor.matmul(
                            pss[n], s2_sb[:, m, ksl], act1[:, ksl, ts(n, NB)],
                            start=(kk == 0), stop=(kk == HC // 2 - 1),
                            perf_mode=DR,
                        )
                for n in range(NSPLIT):
                    nc.scalar.activation(
                        act2[:, m, ts(n, NB)], pss[n], AF.Sign, bias=b2v[:, m:m + 1]
                    )

            # ---- fc3: fp8 DoubleRow, bn affine + hardtanh -> act3 (DVE) ----
            for m in range(HC):
                pss = [psum.tile([128, NB], f32, tag="mm", name="ps") for _ in range(NSPLIT)]
                for kk in range(HC // 2):
                    ksl = slice(2 * kk, 2 * kk + 2)
                    for n in range(NSPLIT):
                        nc.tensor.matmul(
                            pss[n], s3_sb[:, m, ksl], act2[:, ksl, ts(n, NB)],
                            start=(kk == 0), stop=(kk == HC // 2 - 1),
                            perf_mode=DR,
                        )
                for n in range(NSPLIT):
                    t = tmp.tile([128, NB], f32, tag="t3")
                    nc.scalar.activation(
                        t, pss[n], AF.Identity,
                        bias=sh3v[:, m:m + 1], scale=sc3v[:, m:m + 1],
                    )
                    nc.vector.tensor_scalar(
                        out=act3[:, m, ts(n, NB)], in0=t,
                        scalar1=-1.0, scalar2=1.0,
                        op0=ALU.max, op1=ALU.min,
                    )

            # ---- fc4 + log_softmax ----
            # ltr[p, 32u+o] = logit class o of column 32u+p; column 32u+p
            # carries batch row 64p+u (host permutation), so partition p of
            # outf holds y rows 64p..64p+63 contiguously. Phase 1 per n
            # (through Exp/reduce) overlaps later fc4 matmuls; the Ln's
            # (different act table than Exp) run batched at the end.
            yb = y_t[:]
            b4r = b4bc[:]
            NB4 = 256                     # fc4 column block (shorter tail)
            NJ4 = NB4 // 32
            for nn in range(BS // NB4):
                ps4 = psum.tile([32, NB4], f32, tag="mm", name="ps4")
                for k in range(HC):
                    nc.tensor.matmul(
                        ps4, w4_sb[:, k], act3[:, k, ts(nn, NB4)],
                        start=(k == 0), stop=(k == HC - 1),
                    )
                nc.vector.transpose(ltr[:, ts(nn, NB4)], ps4)
                base = ltr[:, ts(nn, NB4)]
                ltv = bass.AP(tensor=base.tensor, offset=base.offset,
                              ap=[base.ap[0], [32, NJ4], [1, OUT_F]])
                nc.vector.tensor_tensor(
                    out=ltv, in0=ltv,
                    in1=bass.AP(tensor=b4r.tensor, offset=b4r.offset,
                                ap=[[b4r.ap[0][0], 32], [0, NJ4], b4r.ap[1]]),
                    op=ALU.add,
                )
                nc.scalar.activation(es[:, ts(nn, NJ4), :], ltv, AF.Exp)
                nc.vector.tensor_reduce(
                    out=lse[:, ts(nn, NJ4)], in_=es[:, ts(nn, NJ4), :],
                    axis=mybir.AxisListType.X, op=ALU.add,
                )
            nc.scalar.activation(lse, lse, AF.Ln)
            # one full-width subtract + one contiguous y DMA (32 x 2560B)
            lall = ltr[:]
            ltv_all = bass.AP(tensor=lall.tensor, offset=lall.offset,
                              ap=[lall.ap[0], [32, NSPLIT * NJ], [1, OUT_F]])
            lser = lse[:]
            nc.vector.tensor_tensor(
                out=outf, in0=ltv_all,
                in1=bass.AP(tensor=lser.tensor, offset=lser.offset,
                            ap=[lser.ap[0], lser.ap[1], [0, OUT_F]]),
                op=ALU.subtract,
            )
            nc.sync.dma_start(
                out=bass.AP(tensor=yb.tensor, offset=yb.offset,
                            ap=[[64 * OUT_F, 32], [OUT_F, NSPLIT * NJ],
                                [1, OUT_F]]),
                in_=outf,
            )

    nc.finalize()
    return nc


def _host_prep(inputs):
    """Shard x, binarize/lay out weights (partition-major), fold bn biases."""
    import ml_dtypes

    f16 = np.float16
    f8 = ml_dtypes.float8_e4m3

    x = np.asarray(inputs["x"], np.float32)
    w1 = np.asarray(inputs["w1"], np.float32)
    w2 = np.asarray(inputs["w2"], np.float32)
    w3 = np.asarray(inputs["w3"], np.float32)
    w4 = np.asarray(inputs["w4"], np.float32)
    b1 = np.asarray(inputs["b1"], np.float32)
    b2 = np.asarray(inputs["b2"], np.float32)
    b3 = np.asarray(inputs["b3"], np.float32)
    b4 = np.asarray(inputs["b4"], np.float32)

    EPS = np.float64(1e-5)

    def gv(i):
        return (np.asarray(inputs[f"g{i}"], np.float32),
                np.asarray(inputs[f"be{i}"], np.float32),
                np.asarray(inputs[f"m{i}"], np.float32),
                np.asarray(inputs[f"v{i}"], np.float32))

    g1, be1, m1, v1 = gv(1)
    g2, be2, m2, v2 = gv(2)
    g3, be3, m3, v3 = gv(3)
    # sign(bn(h)) == sign(h + (b - m)) requires gamma > 0 and beta == 0
    assert np.all(g1 > 0) and np.all(be1 == 0), "unsupported bn1 params"
    assert np.all(g2 > 0) and np.all(be2 == 0), "unsupported bn2 params"

    def pmaj(v):  # [1024] -> [128, 8] partition-major
        return np.ascontiguousarray(v.reshape(HC, 128).T)

    bias1 = pmaj(((b1 - m1) * 64.0).astype(np.float32))  # fc1 runs at 2^6
    bias2 = pmaj((b2 - m2).astype(np.float32))
    r3 = 1.0 / np.sqrt(v3.astype(np.float64) + EPS)
    sc3 = pmaj((r3 * g3).astype(np.float32))
    sh3 = pmaj(((b3 - m3).astype(np.float64) * r3 * g3 + be3).astype(np.float32))

    # fc1 weights: rows 0-767 split [p][m][k][c]; hi at sign*2^6 (f16),
    # lo at sign*2^-6 (fp8, min normal). Hi chunk 6 is the packed tail:
    # rows 768-783 replicated at p0-15 and p16-31 (hi/lo), zero elsewhere.
    s1f = np.sign(w1).T.astype(np.float32)              # [784, 1024]
    body = s1f[:768].reshape(6, 128, HC, 128).transpose(2, 0, 1, 3)  # [m,k,p,c]
    s1h = np.zeros((HC, KH, 128, 128), np.float32)
    s1h[:, :6] = body * 64.0
    tail = s1f[768:IN_F].reshape(16, HC, 128) * 64.0    # [16, 8, 128]
    for mm in range(HC):
        s1h[mm, 6, 0:16] = tail[:, mm]
        s1h[mm, 6, 16:32] = tail[:, mm]
    s1h = np.ascontiguousarray(s1h.transpose(2, 0, 1, 3)).astype(f8)  # [p,m,k,c]
    s1l = np.ascontiguousarray(
        (body * (2.0 ** -6)).transpose(2, 0, 1, 3)).astype(f8)         # [p,m,k,c]

    def wlay(w, dt):  # [out, in] -> [p, m, k, c] partition-major
        st = np.sign(w).T.astype(np.float32)            # [in, out]
        a = st.reshape(HC, 128, HC, 128).transpose(2, 0, 1, 3)  # [m,k,p,c]
        return np.ascontiguousarray(a.transpose(2, 0, 1, 3)).astype(dt)

    s2t = wlay(w2, f8)
    s3t = wlay(w3, f8)
    w4p = np.zeros((H, 32), np.float32)                 # zero-pad 10 -> 32
    w4p[:, :OUT_F] = w4.T
    w4t = np.ascontiguousarray(
        w4p.astype(f16).reshape(HC, 128, 32).transpose(1, 0, 2))  # [p,m,o]

    shared = dict(s1h=s1h, s1l=s1l, s2t=s2t, s3t=s3t, w4t=w4t,
                  bias1=bias1, bias2=bias2, sc3=sc3, sh3=sh3, b4=b4)

    # column permutation: kernel column c carries batch row 64*(c%32)+c//32,
    # so the transposed output lands contiguously per partition.
    cc = np.arange(BS)
    perm = 64 * (cc % 32) + cc // 32

    in_maps = []
    for c in range(N_CORES):
        xs = x[c * BS:(c + 1) * BS][perm]               # [2048, 784] permuted
        xcols = np.ascontiguousarray(xs.T)              # [784, 2048] fp32
        xhi = xcols.astype(f16)
        lo32 = xcols - xhi.astype(np.float32)           # exact residual
        xh = np.zeros((KH, 128, BS), f16)
        xh[:6] = xhi[:768].reshape(6, 128, BS)
        xh[6, 0:16] = xhi[768:IN_F]
        xh[6, 16:32] = lo32[768:IN_F].astype(f16)
        # [k, p, (n nb)] -> [p, n, k, nb]
        xh = np.ascontiguousarray(
            xh.reshape(KH, 128, NSPLIT, NB).transpose(1, 2, 0, 3))
        xl = np.ascontiguousarray(
            (lo32[:768] * 4096.0).reshape(KL, 128, NSPLIT, NB)
            .transpose(1, 2, 0, 3)).astype(f8)
        m = dict(shared)
        m["xh"] = xh
        m["xl"] = xl
        in_maps.append(m)
    return in_maps


def kernel(**inputs):
    global LAST_RESULT
    from concourse.bass_utils import run_bass_kernel_spmd

    if "nc" not in _PLAN:
        _PLAN["nc"] = _build_nc()
    nc = _PLAN["nc"]

    in_maps = _host_prep(inputs)
    br = run_bass_kernel_spmd(
        nc, in_maps, list(range(N_CORES)),
        tmpdir=os.environ.get("KERNEL_TMPDIR") or None,
    )
    LAST_RESULT = br
    out = np.concatenate([br.results[c]["y"] for c in range(N_CORES)], axis=0)
    return out.astype(np.float32)
